# revision 1
# baseline (speedup 1.0000x reference)
"""MidGCN forward on 8 Trainium2 NeuronCores (Bass/Tile, SPMD row-sharding).

Math (alpha = 0.5):
  DAD   = d_row * adj * d_col          (d = rsqrt of row/col sums)
  adj_f = (0.5*I - DAD)(I + DAD) = 0.5*I - 0.5*DAD - DAD@DAD
  h     = relu(adj_f @ (x @ W1))
  out   = log_softmax(adj_f @ (h @ W2) + b2)

Key rewrite: never materialize adj_f / DAD@DAD.  With
adjC = adj * d_col (folded into the resident slab once) and
P(y) = adjC @ y, every application is DAD@y = d_row * P(y), so
  adj_f @ y = 0.5*y - d_row*(0.5*P(y) + P(d_row*P(y)))
and each P() is an adjC @ (narrow) matmul.

Sharding: core i holds rows_i = [1024*i, 1024*(i+1)) of adj as the
host-transposed slab adjT_i = adj[rows_i, :].T ([8192, 1024] bf16),
resident in SBUF for all four passes.  Column sums: per-core partials
(free-axis reduces split DVE/ACT, hidden under the DMA load) +
AllReduce; the full d_col then scales the slab in place (global tile
index -> no per-core addressing).  Row sums: ones-vector PE pass over
the raw slab, also hidden under the load (and it warms the PE).  The
x@W1 shard is gathered raw (bf16) while the slab still loads, so pass
1 starts right after the AllReduce.  Narrow activations are scaled
shard-wise (d_row only, purely local) and AllGathered between passes.
Dummy matmul chains bridge the collective gaps to keep the PE HAM
un-throttled.  Output: each core computes log-softmax on its own
[1024, 2] rows; the host concatenates.
"""

import numpy as np
import ml_dtypes

NCORE = 8
N = 8192
NF = 512
NH = 256
NC = 2
RPC = N // NCORE          # rows per core = 1024
KT = N // 128             # 64 contraction tiles
MT = RPC // 128           # 8 output row tiles per core
FT = NF // 128            # 4 k-tiles for x @ W1

_CACHE = {}


def _build(lite=False, sim=False):
    import concourse.bass as bass
    import concourse.mybir as mybir
    import concourse.tile as tile
    from concourse import bacc, masks
    from concourse.bass import ts

    BF = mybir.dt.bfloat16
    F32 = mybir.dt.float32
    AX = mybir.AxisListType
    OP = mybir.AluOpType
    AF = mybir.ActivationFunctionType

    nc = bacc.Bacc("TRN2", target_bir_lowering=False, debug=False,
                   num_devices=NCORE)

    adjT = nc.dram_tensor("adjT", [N, RPC], BF, kind="ExternalInput")
    xT = nc.dram_tensor("xT", [NF, RPC], BF, kind="ExternalInput")
    w1 = nc.dram_tensor("w1", [NF, NH], BF, kind="ExternalInput")
    w2h = nc.dram_tensor("w2h", [NH, NC], BF, kind="ExternalInput")
    b2 = nc.dram_tensor("b2", [1, NC], F32, kind="ExternalInput")
    out = nc.dram_tensor("out", [RPC, NC], F32, kind="ExternalOutput")

    cs_in = nc.dram_tensor("cs_in", [N], F32)
    cs_ar = nc.dram_tensor("cs_ar", [N], F32, addr_space="Shared")
    rs_dram = nc.dram_tensor("rs_dram", [RPC], F32)
    zs_in = nc.dram_tensor("zs_in", [RPC, NH], BF)
    zs_out = nc.dram_tensor("zs_out", [N, NH], BF, addr_space="Shared")
    zt_in = nc.dram_tensor("zt_in", [RPC, NH], BF)
    zt_out = nc.dram_tensor("zt_out", [N, NH], BF, addr_space="Shared")
    zv_in = nc.dram_tensor("zv_in", [RPC, NC], BF)
    zv_out = nc.dram_tensor("zv_out", [N, NC], BF, addr_space="Shared")
    zu_in = nc.dram_tensor("zu_in", [RPC, NC], BF)
    zu_out = nc.dram_tensor("zu_out", [N, NC], BF, addr_space="Shared")
    RG = [list(range(NCORE))]

    if lite:
        # I/O-identical null kernel: measures tunnel/dispatch overhead.
        with tile.TileContext(nc) as tc:
            with tc.tile_pool(name="p0", bufs=1) as p0:
                o = p0.tile([128, MT, NC], F32, tag="o")
                nc.vector.memset(o, 0.0)
                nc.sync.dma_start(
                    out=out[:].rearrange("(mt p) c -> p mt c", p=128), in_=o)
        nc.compile()
        return nc

    with tile.TileContext(nc) as tc:
        from contextlib import ExitStack
        with ExitStack() as ctx:
            p_adj = ctx.enter_context(tc.tile_pool(name="p_adj", bufs=KT))
            p_zb = ctx.enter_context(tc.tile_pool(name="p_zb", bufs=KT))
            p_one = ctx.enter_context(tc.tile_pool(name="p_one", bufs=1))
            p_rot = ctx.enter_context(tc.tile_pool(name="p_rot", bufs=2))

            # ---------- persistent SBUF ----------
            csp = p_one.tile([128, KT], F32, tag="csp")
            s_sb = p_one.tile([128, MT, NH], F32, tag="s")
            xT_sb = p_one.tile([128, FT, RPC], BF, tag="xT")
            w1_sb = p_one.tile([128, FT, NH], BF, tag="w1")
            w2_sb = p_one.tile([128, NC, NC], BF, tag="w2")
            b2_sb = p_one.tile([128, NC], F32, tag="b2")
            ident = p_one.tile([128, 128], BF, tag="ident")
            ones_sb = p_one.tile([128, 1], BF, tag="ones")
            dcolf = p_one.tile([128, KT], F32, tag="dcolf")
            row_sb = p_one.tile([1, RPC], F32, tag="rowsb")
            rloc = p_one.tile([128, MT], F32, tag="rloc")
            drow = p_one.tile([128, MT], F32, tag="drow")
            n2dr = p_one.tile([128, MT], F32, tag="n2dr")
            ndr = p_one.tile([128, MT], F32, tag="ndr")
            vh_sb = p_one.tile([128, MT, NC], F32, tag="vh")
            usb = p_one.tile([128, MT, NC], F32, tag="usb")
            zv_sb = p_one.tile([128, MT, NC], BF, tag="zvs")
            zu_sb = p_one.tile([128, MT, NC], BF, tag="zus")
            zvf = p_one.tile([128, KT, NC], BF, tag="zvf")
            zuf = p_one.tile([128, KT, NC], BF, tag="zuf")
            out_sb = p_one.tile([128, MT, NC], F32, tag="osb")

            masks.make_identity(nc, ident)
            nc.vector.memset(ones_sb, 1.0)
            nc.sync.dma_start(out=xT_sb, in_=xT[:].rearrange(
                "(kt p) m -> p kt m", p=128))
            nc.sync.dma_start(out=w1_sb, in_=w1[:].rearrange(
                "(kt p) n -> p kt n", p=128))
            nc.sync.dma_start(out=w2_sb, in_=w2h[:].rearrange(
                "(kt p) n -> p kt n", p=128))
            nc.sync.dma_start(out=b2_sb, in_=b2[:].to_broadcast([128, NC]))

            # ---------- adj slab load; colsum partials on DVE/ACT ----------
            adj_t = []
            for kt in range(KT):
                a = p_adj.tile([128, RPC], BF, tag="adj", name=f"adj{kt}")
                nc.sync.dma_start(out=a, in_=adjT[ts(kt, 128), :])
                if kt % 2 == 0:
                    nc.vector.tensor_reduce(out=csp[:, kt:kt + 1], in_=a,
                                            axis=AX.X, op=OP.add)
                else:
                    scr = p_rot.tile([128, RPC], BF, tag="scr_a",
                                     name=f"scra{kt}")
                    nc.scalar.activation(out=scr, in_=a, func=AF.Copy,
                                         accum_out=csp[:, kt:kt + 1])
                adj_t.append(a)

            with ExitStack() as c1:
                ps_s = c1.enter_context(
                    tc.tile_pool(name="ps_s", bufs=2, space="PSUM"))
                ps_row = c1.enter_context(
                    tc.tile_pool(name="ps_row", bufs=2, space="PSUM"))
                ps_w0 = c1.enter_context(
                    tc.tile_pool(name="ps_w0", bufs=1, space="PSUM"))

                # ---- s = x @ W1; gather it raw (bf16) while slab loads ----
                for mt in range(MT):
                    ps = ps_s.tile([128, NH], F32, tag="ps")
                    for kt in range(FT):
                        nc.tensor.matmul(ps, xT_sb[:, kt, ts(mt, 128)],
                                         w1_sb[:, kt, :],
                                         start=kt == 0, stop=kt == FT - 1)
                    nc.scalar.activation(out=s_sb[:, mt, :], in_=ps,
                                         func=AF.Copy)
                    zs_t = p_rot.tile([128, NH], BF, tag="zs", bufs=4)
                    nc.vector.tensor_copy(zs_t, ps)
                    nc.sync.dma_start(out=zs_in[ts(mt, 128), :], in_=zs_t)
                if sim:
                    nc.sync.dma_start(out=zs_out[0:RPC, :], in_=zs_in[:])
                else:
                    nc.gpsimd.collective_compute(
                        "AllGather", OP.bypass, replica_groups=RG,
                        ins=[zs_in[:]], outs=[zs_out[:]])
                zb_t = []
                for kt in range(KT):
                    z = p_zb.tile([128, NH], BF, tag="zb", name=f"zb{kt}")
                    nc.sync.dma_start(out=z, in_=zs_out[ts(kt, 128), :])
                    zb_t.append(z)

                # ---- row sums: ones-vector PE pass over the raw slab ----
                prow = [ps_row.tile([1, 512], F32, tag="pr", name=f"pr{j}")
                        for j in range(2)]
                for kt in range(KT):
                    for j in range(2):
                        nc.tensor.matmul(prow[j], ones_sb,
                                         adj_t[kt][:, ts(j, 512)],
                                         start=kt == 0, stop=kt == KT - 1)
                for j in range(2):
                    nc.vector.tensor_copy(row_sb[0:1, ts(j, 512)], prow[j])
                nc.sync.dma_start(out=rs_dram[:], in_=row_sb[0:1, :])
                nc.sync.dma_start(
                    out=rloc,
                    in_=rs_dram[:].rearrange("(mt p) -> p mt", p=128))
                nc.scalar.activation(out=drow, in_=rloc, func=AF.Sqrt)
                nc.vector.reciprocal(drow, drow)
                nc.vector.tensor_scalar_mul(n2dr, drow, -2.0)
                nc.vector.tensor_scalar_mul(ndr, drow, -1.0)

                # keep PE warm while the colsum AllReduce runs
                pw = ps_w0.tile([128, 512], F32, tag="pw")
                for i in range(24):
                    nc.tensor.matmul(pw, ident, adj_t[KT - 1][:, 0:512],
                                     start=i == 0, stop=i == 23,
                                     skip_group_check=True)

                # ---- colsum AllReduce -> full d_col -> fold into slab ----
                nc.sync.dma_start(
                    out=cs_in[:].rearrange("(kt p) -> p kt", p=128), in_=csp)
                if sim:
                    nc.sync.dma_start(out=cs_ar[:], in_=cs_in[:])
                else:
                    nc.gpsimd.collective_compute(
                        "AllReduce", OP.add, replica_groups=RG,
                        ins=[cs_in[:]], outs=[cs_ar[:]])
                nc.sync.dma_start(
                    out=dcolf,
                    in_=cs_ar[:].rearrange("(kt p) -> p kt", p=128))
                nc.scalar.activation(out=dcolf, in_=dcolf, func=AF.Sqrt)
                nc.vector.reciprocal(dcolf, dcolf)
                for kt in range(KT):
                    nc.vector.tensor_scalar(adj_t[kt], adj_t[kt],
                                            dcolf[:, kt:kt + 1], None,
                                            op0=OP.mult)

            # ---------- passes 1 & 2 and layer-1 epilogue ----------
            with ExitStack() as c2:
                pm = c2.enter_context(
                    tc.tile_pool(name="pm", bufs=4, space="PSUM"))
                ptr = c2.enter_context(
                    tc.tile_pool(name="ptr", bufs=2, space="PSUM"))
                pv = c2.enter_context(
                    tc.tile_pool(name="pv", bufs=1, space="PSUM"))
                pwm = c2.enter_context(
                    tc.tile_pool(name="pwm", bufs=1, space="PSUM"))

                # pass 1: t' = adjC @ zs
                for g in range(2):
                    mts = range(4 * g, 4 * g + 4)
                    pst = {mt: pm.tile([128, NH], F32, tag="pm",
                                       name=f"pst{mt}") for mt in mts}
                    for kt in range(KT):
                        for mt in mts:
                            nc.tensor.matmul(
                                pst[mt], adj_t[kt][:, ts(mt, 128)], zb_t[kt],
                                start=kt == 0, stop=kt == KT - 1)
                    for mt in mts:
                        # T = d_row * t' ; zt = bf16(T) ; A = s - T (in s_sb)
                        T_t = p_rot.tile([128, NH], F32, tag="T", bufs=4)
                        nc.vector.tensor_scalar(T_t, pst[mt],
                                                drow[:, mt:mt + 1], None,
                                                op0=OP.mult)
                        zt_t = p_rot.tile([128, NH], BF, tag="zt", bufs=4)
                        nc.vector.tensor_copy(zt_t, T_t)
                        nc.sync.dma_start(out=zt_in[ts(mt, 128), :],
                                          in_=zt_t)
                        nc.vector.tensor_sub(s_sb[:, mt, :], s_sb[:, mt, :],
                                             T_t)
                        zt_last = zt_t

                # PE warmth across the zt AllGather, anchored to the last
                # zt tile so the chain spans the collective window
                pw1 = pwm.tile([128, 512], F32, tag="pwm", name="pw1")
                nc.tensor.matmul(pw1[:, 0:NH], ident, zt_last,
                                 start=True, stop=False,
                                 skip_group_check=True)
                for i in range(43):
                    nc.tensor.matmul(pw1, ident, adj_t[0][:, 0:512],
                                     start=False, stop=i == 42,
                                     skip_group_check=True)

                if sim:
                    nc.sync.dma_start(out=zt_out[0:RPC, :], in_=zt_in[:])
                else:
                    nc.gpsimd.collective_compute(
                        "AllGather", OP.bypass, replica_groups=RG,
                        ins=[zt_in[:]], outs=[zt_out[:]])
                zb2_t = []
                for kt in range(KT):
                    z = p_zb.tile([128, NH], BF, tag="zb", name=f"zb2_{kt}")
                    nc.sync.dma_start(out=z, in_=zt_out[ts(kt, 128), :])
                    zb2_t.append(z)

                # pass 2: r' = adjC @ zt ; h' = relu(A - 2*d_row*r')
                # v = h' @ (W2/2) accumulated via per-tile PE transposes
                for g in range(2):
                    mts = range(4 * g, 4 * g + 4)
                    psr = {mt: pm.tile([128, NH], F32, tag="pm",
                                       name=f"psr{mt}") for mt in mts}
                    for kt in range(KT):
                        for mt in mts:
                            nc.tensor.matmul(
                                psr[mt], adj_t[kt][:, ts(mt, 128)],
                                zb2_t[kt],
                                start=kt == 0, stop=kt == KT - 1)
                    for mt in mts:
                        B_t = p_rot.tile([128, NH], F32, tag="B", bufs=4)
                        nc.vector.tensor_scalar(B_t, psr[mt],
                                                n2dr[:, mt:mt + 1], None,
                                                op0=OP.mult)
                        nc.vector.tensor_add(B_t, B_t, s_sb[:, mt, :])
                        hp_t = p_rot.tile([128, NH], BF, tag="hp", bufs=4)
                        nc.vector.tensor_scalar_max(hp_t, B_t, 0.0)
                        psv = pv.tile([128, NC], F32, tag="pv")
                        for kh in range(2):
                            pstr = ptr.tile([128, 128], BF, tag="ptr")
                            nc.tensor.transpose(pstr, hp_t[:, ts(kh, 128)],
                                                ident)
                            hT_t = p_rot.tile([128, 128], BF, tag="hT",
                                              bufs=3)
                            nc.scalar.activation(out=hT_t, in_=pstr,
                                                 func=AF.Copy)
                            nc.tensor.matmul(psv, hT_t, w2_sb[:, kh, :],
                                             start=kh == 0, stop=kh == 1)
                        nc.scalar.activation(out=vh_sb[:, mt, :], in_=psv,
                                             func=AF.Copy, scale=0.5)
                        nc.vector.tensor_copy(zv_sb[:, mt, :], psv)

                # PE warmth across the zv AllGather, anchored to zv_sb
                pw2 = pwm.tile([128, 512], F32, tag="pwm", name="pw2")
                nc.tensor.matmul(pw2[:, 0:MT * NC], ident,
                                 zv_sb[:].rearrange("p a b -> p (a b)"),
                                 start=True, stop=False,
                                 skip_group_check=True)
                for i in range(39):
                    nc.tensor.matmul(pw2, ident, adj_t[0][:, 0:512],
                                     start=False, stop=i == 38,
                                     skip_group_check=True)

                nc.sync.dma_start(
                    out=zv_in[:].rearrange("(mt p) c -> p mt c", p=128),
                    in_=zv_sb)

            # ---------- layer-2 narrow passes ----------
            if sim:
                nc.sync.dma_start(out=zv_out[0:RPC, :], in_=zv_in[:])
            else:
                nc.gpsimd.collective_compute(
                    "AllGather", OP.bypass, replica_groups=RG,
                    ins=[zv_in[:]], outs=[zv_out[:]])
            nc.sync.dma_start(
                out=zvf, in_=zv_out[:].rearrange("(kt p) c -> p kt c", p=128))

            with ExitStack() as c3:
                puw = c3.enter_context(
                    tc.tile_pool(name="puw", bufs=6, space="PSUM"))
                pwn = c3.enter_context(
                    tc.tile_pool(name="pwn", bufs=1, space="PSUM"))

                # u' = adjC @ zv
                for g in range(2):
                    mts = range(4 * g, 4 * g + 4)
                    psu = {mt: puw.tile([128, NC], F32, tag="pu",
                                        name=f"psu{mt}") for mt in mts}
                    for kt in range(KT):
                        for mt in mts:
                            nc.tensor.matmul(
                                psu[mt], adj_t[kt][:, ts(mt, 128)],
                                zvf[:, kt, :],
                                start=kt == 0, stop=kt == KT - 1)
                    for mt in mts:
                        nc.vector.tensor_scalar_mul(usb[:, mt, :], psu[mt],
                                                    0.5)
                        nc.vector.tensor_scalar(zu_sb[:, mt, :], psu[mt],
                                                drow[:, mt:mt + 1], None,
                                                op0=OP.mult)

                # PE warmth across the zu AllGather, anchored to zu_sb
                pw3 = pwn.tile([128, 512], F32, tag="pwn", name="pw3")
                nc.tensor.matmul(pw3[:, 0:MT * NC], ident,
                                 zu_sb[:].rearrange("p a b -> p (a b)"),
                                 start=True, stop=False,
                                 skip_group_check=True)
                for i in range(39):
                    nc.tensor.matmul(pw3, ident, adj_t[0][:, 0:512],
                                     start=False, stop=i == 38,
                                     skip_group_check=True)

                nc.sync.dma_start(
                    out=zu_in[:].rearrange("(mt p) c -> p mt c", p=128),
                    in_=zu_sb)
                if sim:
                    nc.sync.dma_start(out=zu_out[0:RPC, :], in_=zu_in[:])
                else:
                    nc.gpsimd.collective_compute(
                        "AllGather", OP.bypass, replica_groups=RG,
                        ins=[zu_in[:]], outs=[zu_out[:]])
                nc.sync.dma_start(
                    out=zuf,
                    in_=zu_out[:].rearrange("(kt p) c -> p kt c", p=128))

                # w' = adjC @ zu ; out = logsoftmax(0.5v - drow(0.5u'+w')+b2)
                for g in range(2):
                    mts = range(4 * g, 4 * g + 4)
                    psw = {mt: puw.tile([128, NC], F32, tag="pu",
                                        name=f"psw{mt}") for mt in mts}
                    for kt in range(KT):
                        for mt in mts:
                            nc.tensor.matmul(
                                psw[mt], adj_t[kt][:, ts(mt, 128)],
                                zuf[:, kt, :],
                                start=kt == 0, stop=kt == KT - 1)
                    G_ts, sm_ts = {}, {}
                    for mt in mts:
                        G_t = p_rot.tile([128, NC], F32, tag="G",
                                         name=f"G{mt}", bufs=4)
                        nc.vector.tensor_add(G_t, usb[:, mt, :], psw[mt])
                        nc.vector.tensor_scalar_mul(G_t, G_t,
                                                    ndr[:, mt:mt + 1])
                        nc.vector.tensor_add(G_t, G_t, vh_sb[:, mt, :])
                        nc.vector.tensor_add(G_t, G_t, b2_sb)
                        mx_t = p_rot.tile([128, 1], F32, tag="mx")
                        nc.vector.tensor_reduce(out=mx_t, in_=G_t,
                                                axis=AX.X, op=OP.max)
                        nc.vector.tensor_scalar(G_t, G_t, mx_t, None,
                                                op0=OP.subtract)
                        G_ts[mt] = G_t
                    for mt in mts:
                        ex_t = p_rot.tile([128, NC], F32, tag="ex")
                        sm_t = p_rot.tile([128, 1], F32, tag="sm",
                                          name=f"sm{mt}", bufs=4)
                        nc.scalar.activation(out=ex_t, in_=G_ts[mt],
                                             func=AF.Exp, accum_out=sm_t)
                        sm_ts[mt] = sm_t
                    for mt in mts:
                        lg_t = p_rot.tile([128, 1], F32, tag="lg")
                        nc.scalar.activation(out=lg_t, in_=sm_ts[mt],
                                             func=AF.Ln)
                        nc.vector.tensor_scalar(out_sb[:, mt, :], G_ts[mt],
                                                lg_t, None, op0=OP.subtract)
                nc.sync.dma_start(
                    out=out[:].rearrange("(mt p) c -> p mt c", p=128),
                    in_=out_sb)

    nc.compile()
    return nc


def _get_nc(lite=False):
    key = "nc_lite" if lite else "nc"
    if key not in _CACHE:
        _CACHE[key] = _build(lite=lite)
    return _CACHE[key]


def _prep_in_maps(x, adj, W1, W2, b2):
    bf = ml_dtypes.bfloat16
    f32 = np.float32
    x = np.asarray(x, f32)
    adj = np.asarray(adj, f32)
    w1 = np.asarray(W1, f32).astype(bf)
    w2h = (0.5 * np.asarray(W2, f32)).astype(bf)
    b2v = np.asarray(b2, f32).reshape(1, NC)
    in_maps = []
    for i in range(NCORE):
        rows = slice(i * RPC, (i + 1) * RPC)
        in_maps.append({
            "adjT": adj[rows, :].T.astype(bf),   # one fused copy+cast
            "xT": x[rows, :].T.astype(bf),
            "w1": w1, "w2h": w2h, "b2": b2v,
        })
    return in_maps


def _run(x, adj, W1, W2, b2, trace=False, lite=False, in_maps=None):
    from concourse.bass_utils import run_bass_kernel_spmd
    nc = _get_nc(lite=lite)
    if in_maps is None:
        in_maps = _prep_in_maps(x, adj, W1, W2, b2)
    res = run_bass_kernel_spmd(nc, in_maps, core_ids=list(range(NCORE)),
                               trace=trace)
    out = np.concatenate([r["out"] for r in res.results], axis=0)
    return out, res


def kernel(x, adj, W1, W2, b2):
    out, _ = _run(x, adj, W1, W2, b2, trace=False)
    return out



# revision 10
# speedup vs baseline: 1.5242x; 1.5242x over previous
"""MidGCN forward on 8 Trainium2 NeuronCores (Bass/Tile, SPMD row-sharding).

Math (alpha = 0.5):
  DAD   = d_row * adj * d_col          (d = rsqrt of row/col sums)
  adj_f = (0.5*I - DAD)(I + DAD) = 0.5*I - 0.5*DAD - DAD@DAD
  h     = relu(adj_f @ (x @ W1))
  out   = log_softmax(adj_f @ (h @ W2) + b2)

Strategy vs the bf16 predecessor (269.5us):
  * The adj slab is resident in SBUF as fp8e4 (e4m3): halves the HBM
    load (8MB) and enables DoubleRow matmuls (two 128-deep k-tiles per
    instruction at 0.5 cycles/row): the two NxN passes drop ~4x.
  * d_col is folded into the narrow activations (z's), never into the
    slab: z_q = fp8(S * d_col * z) per pass, so the slab stays raw and
    single-quantized.  Scales S1/S2/Sv/Su keep each z at sigma ~0.7 in
    e4m3 range; the inverse scales fold into the per-row epilogue
    scalars.  Verified numerically: rel err ~5.7e-3 (tolerance 2e-2).
  * Colsum partials ride the slab DMA on DVE/ACT; rowsums via a
    DoubleRow fp8 ones-vector PE pass.  ReduceScatter (not AllReduce)
    delivers exactly the own-row d_col slice - no per-core addressing.
  * rsqrt = Exp(-0.5*Ln(x)): every ACT func used ({Copy,Relu,Exp,Ln})
    lives in one act table -> a single LoadActFuncSet for the kernel.
  * Each pass computes mt-halves 0-3 / 4-7 separately with per-half
    AllGathers; the consumer pass processes the k-parity of the first
    half first, hiding collective+DMA latency under live matmuls.
  * Epilogues use fused scalar_tensor_tensor (one DVE op per AXPY) and
    ACT Copy-with-scale for the fp8 quantizes straight out of PSUM.
"""

import numpy as np
import ml_dtypes

NCORE = 8
N = 8192
NF = 512
NH = 256
NC = 2
RPC = N // NCORE          # rows per core = 1024
KT = N // 128             # 64 contraction tiles
NQ = KT // 2              # 32 DoubleRow k-pairs
MT = RPC // 128           # 8 output row tiles per core
FT = NF // 128            # 4 k-tiles for x @ W1
HR = RPC // 2             # 512 rows per mt-half

S1 = 64.0                 # zq = fp8(S1 * d_col * zs)
S2 = 4096.0               # zt = fp8(psum1 * d_row*d_col*S2/S1)
SV = 16.0                 # zv = fp8(SV * d_col * y2)
SU = 512.0                # zu = fp8(psum_u * d_row*d_col*SU/SV)

_CACHE = {}


def _build(lite=False, sim=False):
    import concourse.bass as bass
    import concourse.mybir as mybir
    import concourse.tile as tile
    from concourse import bacc, masks
    from concourse.bass import ts

    BF = mybir.dt.bfloat16
    F32 = mybir.dt.float32
    FP8 = mybir.dt.float8e4
    AX = mybir.AxisListType
    OP = mybir.AluOpType
    AF = mybir.ActivationFunctionType
    PM = mybir.MatmulPerfMode

    nc = bacc.Bacc("TRN2", target_bir_lowering=False, debug=False,
                   num_devices=NCORE)

    adjT = nc.dram_tensor("adjT", [N, RPC], FP8, kind="ExternalInput")
    xT = nc.dram_tensor("xT", [NF, RPC], BF, kind="ExternalInput")
    w1 = nc.dram_tensor("w1", [NF, NH], BF, kind="ExternalInput")
    w2h = nc.dram_tensor("w2h", [NH, NC], BF, kind="ExternalInput")
    b2 = nc.dram_tensor("b2", [1, NC], F32, kind="ExternalInput")
    out = nc.dram_tensor("out", [RPC, NC], F32, kind="ExternalOutput")

    cs_in = nc.dram_tensor("cs_in", [N], F32)
    cs_rs = nc.dram_tensor("cs_rs", [RPC], F32)
    rs_dram = nc.dram_tensor("rs_dram", [RPC], F32)
    zq_in = nc.dram_tensor("zq_in", [RPC, NH], FP8)
    zq_out = nc.dram_tensor("zq_out", [N, NH], FP8, addr_space="Shared")
    zt_in = nc.dram_tensor("zt_in", [RPC, NH], FP8)
    zt_o = [nc.dram_tensor(f"zt_o{h}", [N // 2, NH], FP8,
                           addr_space="Shared") for h in range(2)]
    zv_in = nc.dram_tensor("zv_in", [RPC, NC], FP8)
    zv_o = [nc.dram_tensor(f"zv_o{h}", [N // 2, NC], FP8,
                           addr_space="Shared") for h in range(2)]
    zu_in = nc.dram_tensor("zu_in", [RPC, NC], FP8)
    zu_o = [nc.dram_tensor(f"zu_o{h}", [N // 2, NC], FP8,
                           addr_space="Shared") for h in range(2)]
    RG = [list(range(NCORE))]

    if lite:
        # I/O-identical null kernel: measures tunnel/dispatch overhead.
        with tile.TileContext(nc) as tc:
            with tc.tile_pool(name="p0", bufs=1) as p0:
                o = p0.tile([128, MT, NC], F32, tag="o")
                nc.vector.memset(o, 0.0)
                nc.sync.dma_start(
                    out=out[:].rearrange("(mt p) c -> p mt c", p=128), in_=o)
        nc.compile()
        return nc

    # k-pair order for passes consuming half-gathered z: the parity-0
    # pairs (kt%8 in 0..3) arrive with AllGather #1, parity-1 with #2.
    QORD = [q for q in range(NQ) if q % 4 < 2] + \
           [q for q in range(NQ) if q % 4 >= 2]

    def hslot(q):
        # (half, flat slot) of k-pair q in the per-half gather layout
        h = 0 if q % 4 < 2 else 1
        return h, 4 * (q // 4) + 2 * (q % 4) - 4 * h

    with tile.TileContext(nc) as tc:
        from contextlib import ExitStack
        with ExitStack() as ctx:
            p_one = ctx.enter_context(tc.tile_pool(name="p_one", bufs=1))
            p_rot = ctx.enter_context(tc.tile_pool(name="p_rot", bufs=2))

            # ---------- persistent SBUF ----------
            slab = p_one.tile([128, KT, RPC], FP8, tag="slab")
            zbq = p_one.tile([128, KT, NH], FP8, tag="zbq")
            zbt = [p_one.tile([128, KT // 2, NH], FP8, tag=f"zbt{h}",
                              name=f"zbt{h}") for h in range(2)]
            zvf = [p_one.tile([128, KT // 2, NC], FP8, tag=f"zvf{h}",
                              name=f"zvf{h}") for h in range(2)]
            zuf = [p_one.tile([128, KT // 2, NC], FP8, tag=f"zuf{h}",
                              name=f"zuf{h}") for h in range(2)]
            xT_sb = p_one.tile([128, FT, RPC], BF, tag="xT")
            w1_sb = p_one.tile([128, FT, NH], BF, tag="w1")
            w2_sb = p_one.tile([128, 2, NC], BF, tag="w2")
            b2_sb = p_one.tile([128, NC], F32, tag="b2")
            ident = p_one.tile([128, 128], BF, tag="ident")
            ones8 = p_one.tile([128, 2, 32], FP8, tag="ones")
            csp = p_one.tile([128, KT], F32, tag="csp")
            s_sb = p_one.tile([128, MT, NH], F32, tag="s")
            zq_sb = p_one.tile([128, MT, NH], FP8, tag="zq")
            zt_sb = p_one.tile([128, MT, NH], FP8, tag="zt")
            zv_sb = p_one.tile([128, MT, NC], FP8, tag="zv")
            zu_sb = p_one.tile([128, MT, NC], FP8, tag="zu")
            y_sb = p_one.tile([128, MT, NC], F32, tag="y")
            row_sb = p_one.tile([1, RPC], F32, tag="rowsb")
            rloc = p_one.tile([128, MT], F32, tag="rloc")
            drow = p_one.tile([128, MT], F32, tag="drow")
            dcl = p_one.tile([128, MT], F32, tag="dcl")
            ddt = p_one.tile([128, MT], F32, tag="ddt")
            q1 = p_one.tile([128, MT], F32, tag="q1")
            dd2 = p_one.tile([128, MT], F32, tag="dd2")
            dvq = p_one.tile([128, MT], F32, tag="dvq")
            du2 = p_one.tile([128, MT], F32, tag="du2")
            e1n = p_one.tile([128, MT], F32, tag="e1n")
            e2 = p_one.tile([128, MT], F32, tag="e2")
            eu = p_one.tile([128, MT], F32, tag="eu")
            ew = p_one.tile([128, MT], F32, tag="ew")
            mx_sb = p_one.tile([128, MT], F32, tag="mx")
            out_sb = p_one.tile([128, MT, NC], F32, tag="osb")

            masks.make_identity(nc, ident)
            nc.vector.memset(ones8, 1.0)

            # weights / x first on the DMA queue, then the slab
            nc.sync.dma_start(out=xT_sb, in_=xT[:].rearrange(
                "(kt p) m -> p kt m", p=128))
            nc.sync.dma_start(out=w1_sb, in_=w1[:].rearrange(
                "(kt p) n -> p kt n", p=128))
            nc.sync.dma_start(out=w2_sb, in_=w2h[:].rearrange(
                "(kh p) c -> p kh c", p=128))
            nc.sync.dma_start(out=b2_sb, in_=b2[:].to_broadcast([128, NC]))

            with ExitStack() as c1:
                pm = c1.enter_context(
                    tc.tile_pool(name="pm", bufs=4, space="PSUM"))

                # ---- s = x @ W1 first in PE program order ----
                for g in range(2):
                    mts = range(4 * g, 4 * g + 4)
                    ps = {mt: pm.tile([128, NH], F32, tag="pm",
                                      name=f"ps{mt}") for mt in mts}
                    for kt in range(FT):
                        for mt in mts:
                            nc.tensor.matmul(ps[mt], xT_sb[:, kt, ts(mt, 128)],
                                             w1_sb[:, kt, :],
                                             start=kt == 0, stop=kt == FT - 1)
                    for mt in mts:
                        if mt % 2 == 0:
                            nc.vector.tensor_copy(s_sb[:, mt, :], ps[mt])
                        else:
                            nc.scalar.activation(out=s_sb[:, mt, :],
                                                 in_=ps[mt], func=AF.Copy)

                # ---- slab load; colsum partials on DVE/ACT; rowsum on PE --
                with ExitStack() as c0:
                    pr_p = c0.enter_context(
                        tc.tile_pool(name="prp", bufs=4, space="PSUM"))
                    prow = [pr_p.tile([32, 256], F32, tag="pr",
                            name=f"pr{j}") for j in range(4)]
                    for c in range(8):
                        nc.sync.dma_start(
                            out=slab[:, 8 * c:8 * c + 8, :],
                            in_=adjT[ts(c, RPC), :].rearrange(
                                "(kt p) m -> p kt m", p=128))
                        for kt in range(8 * c, 8 * c + 8):
                            if kt % 16 < 7:
                                nc.vector.tensor_reduce(
                                    out=csp[:, kt:kt + 1], in_=slab[:, kt, :],
                                    axis=AX.X, op=OP.add)
                            else:
                                scr = p_rot.tile([128, RPC], FP8, tag="scr",
                                                 name=f"scr{kt}")
                                nc.scalar.activation(
                                    out=scr, in_=slab[:, kt, :], func=AF.Copy,
                                    accum_out=csp[:, kt:kt + 1])
                        for q in range(4 * c, 4 * c + 4):
                            for j in range(4):
                                nc.tensor.matmul(
                                    prow[j], ones8,
                                    slab[:, 2 * q:2 * q + 2, ts(j, 256)],
                                    start=q == 0, stop=q == NQ - 1,
                                    perf_mode=PM.DoubleRow)
                    for j in range(4):
                        nc.vector.tensor_copy(row_sb[0:1, ts(j, 256)],
                                              prow[j][0:1, :])

                # d_row = exp(-0.5 ln(rowsum)) in [128, MT] layout
                nc.sync.dma_start(out=rs_dram[:], in_=row_sb[0:1, :])
                nc.sync.dma_start(
                    out=rloc,
                    in_=rs_dram[:].rearrange("(mt p) -> p mt", p=128))
                nc.scalar.activation(out=drow, in_=rloc, func=AF.Ln)
                nc.scalar.activation(out=drow, in_=drow, func=AF.Exp,
                                     scale=-0.5)

                # ---- colsum ReduceScatter -> own-row d_col slice ----
                nc.sync.dma_start(
                    out=cs_in[:].rearrange("(kt p) -> p kt", p=128), in_=csp)
                if sim:
                    nc.sync.dma_start(out=cs_rs[:], in_=cs_in[0:RPC])
                else:
                    nc.gpsimd.collective_compute(
                        "ReduceScatter", OP.add, replica_groups=RG,
                        ins=[cs_in[:]], outs=[cs_rs[:]])
                nc.sync.dma_start(
                    out=dcl, in_=cs_rs[:].rearrange("(mt p) -> p mt", p=128))
                nc.scalar.activation(out=dcl, in_=dcl, func=AF.Ln)
                nc.scalar.activation(out=dcl, in_=dcl, func=AF.Exp,
                                     scale=-0.5)

                # per-row scalar vectors
                nc.vector.tensor_tensor(ddt, drow, dcl, op=OP.mult)
                nc.vector.tensor_scalar_mul(q1, dcl, S1)
                nc.vector.tensor_scalar_mul(dd2, ddt, S2 / S1)
                nc.vector.tensor_scalar_mul(dvq, dcl, SV)
                nc.vector.tensor_scalar_mul(du2, ddt, SU / SV)
                nc.vector.tensor_scalar_mul(e1n, drow, -1.0 / S1)
                nc.vector.tensor_scalar_mul(e2, drow, -2.0 / S2)
                nc.vector.tensor_scalar_mul(eu, drow, -0.5 / SV)
                nc.vector.tensor_scalar_mul(ew, drow, -1.0 / SU)

                # zq = fp8(S1 * d_col * zs); gather; load k-major
                for mt in range(MT):
                    nc.scalar.activation(out=zq_sb[:, mt, :],
                                         in_=s_sb[:, mt, :], func=AF.Copy,
                                         scale=q1[:, mt:mt + 1])
                nc.sync.dma_start(
                    out=zq_in[:].rearrange("(mt p) n -> p mt n", p=128),
                    in_=zq_sb)
                if sim:
                    nc.sync.dma_start(out=zq_out[0:RPC, :], in_=zq_in[:])
                else:
                    nc.gpsimd.collective_compute(
                        "AllGather", OP.bypass, replica_groups=RG,
                        ins=[zq_in[:]], outs=[zq_out[:]])
                for c in range(4):
                    nc.sync.dma_start(
                        out=zbq[:, 16 * c:16 * c + 16, :],
                        in_=zq_out[ts(c, 2048), :].rearrange(
                            "(kt p) n -> p kt n", p=128))

                # ---------- pass 1: psum1 = adj @ zq, by mt-halves ----------
                for hf in range(2):
                    mts = range(4 * hf, 4 * hf + 4)
                    pp = {mt: pm.tile([128, NH], F32, tag="pm",
                                      name=f"p1_{mt}") for mt in mts}
                    for q in range(NQ):
                        for mt in mts:
                            nc.tensor.matmul(
                                pp[mt], slab[:, 2 * q:2 * q + 2, ts(mt, 128)],
                                zbq[:, 2 * q:2 * q + 2, :],
                                start=q == 0, stop=q == NQ - 1,
                                perf_mode=PM.DoubleRow)
                    # zt = fp8(psum1 * dd2) -> store half -> gather half
                    for mt in mts:
                        nc.scalar.activation(out=zt_sb[:, mt, :], in_=pp[mt],
                                             func=AF.Copy,
                                             scale=dd2[:, mt:mt + 1])
                    nc.sync.dma_start(
                        out=zt_in[ts(hf, HR), :].rearrange(
                            "(mt p) n -> p mt n", p=128),
                        in_=zt_sb[:, 4 * hf:4 * hf + 4, :])
                    if sim:
                        nc.sync.dma_start(out=zt_o[hf][0:HR, :],
                                          in_=zt_in[ts(hf, HR), :])
                    else:
                        nc.gpsimd.collective_compute(
                            "AllGather", OP.bypass, replica_groups=RG,
                            ins=[zt_in[ts(hf, HR), :]], outs=[zt_o[hf][:]])
                    nc.sync.dma_start(
                        out=zbt[hf],
                        in_=zt_o[hf][:].rearrange("(f p) n -> p f n", p=128))
                    # A = s - T = s + e1n * psum1 (in place in s_sb)
                    for mt in mts:
                        nc.vector.scalar_tensor_tensor(
                            s_sb[:, mt, :], pp[mt], e1n[:, mt:mt + 1],
                            s_sb[:, mt, :], op0=OP.mult, op1=OP.add)

                # ---------- pass 2 + layer-1 epilogue ----------
                with ExitStack() as c2:
                    ptr = c2.enter_context(
                        tc.tile_pool(name="ptr", bufs=2, space="PSUM"))
                    pv = c2.enter_context(
                        tc.tile_pool(name="pv", bufs=1, space="PSUM"))
                    for hf in range(2):
                        mts = range(4 * hf, 4 * hf + 4)
                        pp = {mt: pm.tile([128, NH], F32, tag="pm",
                                          name=f"p2_{mt}") for mt in mts}
                        for qi, q in enumerate(QORD):
                            for mt in mts:
                                h_, f_ = hslot(q)
                                nc.tensor.matmul(
                                    pp[mt],
                                    slab[:, 2 * q:2 * q + 2, ts(mt, 128)],
                                    zbt[h_][:, f_:f_ + 2, :],
                                    start=qi == 0, stop=qi == NQ - 1,
                                    perf_mode=PM.DoubleRow)
                        for mt in mts:
                            # h2 = relu(A + e2*psum2); y2 = h2 @ (W2/2)
                            h2p = p_rot.tile([128, NH], F32, tag="h2p",
                                             bufs=3)
                            nc.vector.scalar_tensor_tensor(
                                h2p, pp[mt], e2[:, mt:mt + 1], s_sb[:, mt, :],
                                op0=OP.mult, op1=OP.add)
                            hp = p_rot.tile([128, NH], BF, tag="hp", bufs=3)
                            nc.scalar.activation(out=hp, in_=h2p,
                                                 func=AF.Relu)
                            psv = pv.tile([128, NC], F32, tag="pv")
                            for kh in range(2):
                                pstr = ptr.tile([128, 128], BF, tag="ptr")
                                nc.tensor.transpose(pstr, hp[:, ts(kh, 128)],
                                                    ident)
                                hT = p_rot.tile([128, 128], BF, tag="hT",
                                                bufs=3)
                                nc.scalar.activation(out=hT, in_=pstr,
                                                     func=AF.Copy)
                                nc.tensor.matmul(psv, hT, w2_sb[:, kh, :],
                                                 start=kh == 0, stop=kh == 1)
                            nc.scalar.activation(out=y_sb[:, mt, :], in_=psv,
                                                 func=AF.Copy, scale=0.5)
                            nc.scalar.activation(out=zv_sb[:, mt, :], in_=psv,
                                                 func=AF.Copy,
                                                 scale=dvq[:, mt:mt + 1])
                        nc.sync.dma_start(
                            out=zv_in[ts(hf, HR), :].rearrange(
                                "(mt p) c -> p mt c", p=128),
                            in_=zv_sb[:, 4 * hf:4 * hf + 4, :])
                        if sim:
                            nc.sync.dma_start(out=zv_o[hf][0:HR, :],
                                              in_=zv_in[ts(hf, HR), :])
                        else:
                            nc.gpsimd.collective_compute(
                                "AllGather", OP.bypass, replica_groups=RG,
                                ins=[zv_in[ts(hf, HR), :]],
                                outs=[zv_o[hf][:]])
                        nc.sync.dma_start(
                            out=zvf[hf],
                            in_=zv_o[hf][:].rearrange("(f p) c -> p f c",
                                                      p=128))

            # ---------- narrow passes ----------
            with ExitStack() as c3:
                pnar = c3.enter_context(
                    tc.tile_pool(name="pnar", bufs=8, space="PSUM"))

                # pass 3: psum_u = adj @ zv
                for hf in range(2):
                    mts = range(4 * hf, 4 * hf + 4)
                    pu = {mt: pnar.tile([128, NC], F32, tag="pu",
                                        name=f"pu{mt}") for mt in mts}
                    for qi, q in enumerate(QORD):
                        for mt in mts:
                            h_, f_ = hslot(q)
                            nc.tensor.matmul(
                                pu[mt], slab[:, 2 * q:2 * q + 2, ts(mt, 128)],
                                zvf[h_][:, f_:f_ + 2, :],
                                start=qi == 0, stop=qi == NQ - 1,
                                perf_mode=PM.DoubleRow)
                    for mt in mts:
                        nc.scalar.activation(out=zu_sb[:, mt, :], in_=pu[mt],
                                             func=AF.Copy,
                                             scale=du2[:, mt:mt + 1])
                    nc.sync.dma_start(
                        out=zu_in[ts(hf, HR), :].rearrange(
                            "(mt p) c -> p mt c", p=128),
                        in_=zu_sb[:, 4 * hf:4 * hf + 4, :])
                    if sim:
                        nc.sync.dma_start(out=zu_o[hf][0:HR, :],
                                          in_=zu_in[ts(hf, HR), :])
                    else:
                        nc.gpsimd.collective_compute(
                            "AllGather", OP.bypass, replica_groups=RG,
                            ins=[zu_in[ts(hf, HR), :]], outs=[zu_o[hf][:]])
                    nc.sync.dma_start(
                        out=zuf[hf],
                        in_=zu_o[hf][:].rearrange("(f p) c -> p f c",
                                                  p=128))
                    # y += eu * psum_u  (= 0.5*y2 - 0.5*DAD@y2 so far)
                    for mt in mts:
                        nc.vector.scalar_tensor_tensor(
                            y_sb[:, mt, :], pu[mt], eu[:, mt:mt + 1],
                            y_sb[:, mt, :], op0=OP.mult, op1=OP.add)

                # fold the bias in while the gather flies
                for mt in range(MT):
                    nc.vector.tensor_add(y_sb[:, mt, :], y_sb[:, mt, :],
                                         b2_sb)

                # pass 4: psum_w = adj @ zu ; G = y + ew*psum_w; log_softmax
                for hf in range(2):
                    mts = range(4 * hf, 4 * hf + 4)
                    pw = {mt: pnar.tile([128, NC], F32, tag="pu",
                                        name=f"pw{mt}") for mt in mts}
                    for qi, q in enumerate(QORD):
                        for mt in mts:
                            h_, f_ = hslot(q)
                            nc.tensor.matmul(
                                pw[mt], slab[:, 2 * q:2 * q + 2, ts(mt, 128)],
                                zuf[h_][:, f_:f_ + 2, :],
                                start=qi == 0, stop=qi == NQ - 1,
                                perf_mode=PM.DoubleRow)
                    G_ts, sm_ts = {}, {}
                    for mt in mts:
                        G_t = p_rot.tile([128, NC], F32, tag="G",
                                         name=f"G{mt}", bufs=4)
                        nc.vector.scalar_tensor_tensor(
                            G_t, pw[mt], ew[:, mt:mt + 1], y_sb[:, mt, :],
                            op0=OP.mult, op1=OP.add)
                        mx_t = p_rot.tile([128, 1], F32, tag="mx")
                        nc.vector.tensor_reduce(out=mx_t, in_=G_t,
                                                axis=AX.X, op=OP.max)
                        nc.vector.tensor_scalar(G_t, G_t, mx_t, None,
                                                op0=OP.subtract)
                        G_ts[mt] = G_t
                    for mt in mts:
                        ex_t = p_rot.tile([128, NC], F32, tag="ex")
                        sm_t = p_rot.tile([128, 1], F32, tag="sm",
                                          name=f"sm{mt}", bufs=4)
                        nc.scalar.activation(out=ex_t, in_=G_ts[mt],
                                             func=AF.Exp, accum_out=sm_t)
                        sm_ts[mt] = sm_t
                    for mt in mts:
                        lg_t = p_rot.tile([128, 1], F32, tag="lg")
                        nc.scalar.activation(out=lg_t, in_=sm_ts[mt],
                                             func=AF.Ln)
                        nc.vector.tensor_scalar(out_sb[:, mt, :], G_ts[mt],
                                                lg_t, None, op0=OP.subtract)
                nc.sync.dma_start(
                    out=out[:].rearrange("(mt p) c -> p mt c", p=128),
                    in_=out_sb)

    nc.compile()
    return nc


def _get_nc(lite=False):
    key = "nc_lite" if lite else "nc"
    if key not in _CACHE:
        _CACHE[key] = _build(lite=lite)
    return _CACHE[key]


def _prep_in_maps(x, adj, W1, W2, b2):
    bf = ml_dtypes.bfloat16
    e4 = ml_dtypes.float8_e4m3fn
    f32 = np.float32
    x = np.asarray(x, f32)
    adj = np.asarray(adj, f32)
    w1 = np.asarray(W1, f32).astype(bf)
    w2h = (0.5 * np.asarray(W2, f32)).astype(bf)
    b2v = np.asarray(b2, f32).reshape(1, NC)
    in_maps = []
    for i in range(NCORE):
        rows = slice(i * RPC, (i + 1) * RPC)
        in_maps.append({
            "adjT": adj[rows, :].T.astype(e4),   # one fused copy+cast
            "xT": x[rows, :].T.astype(bf),
            "w1": w1, "w2h": w2h, "b2": b2v,
        })
    return in_maps


def _run(x, adj, W1, W2, b2, trace=False, lite=False, in_maps=None):
    from concourse.bass_utils import run_bass_kernel_spmd
    nc = _get_nc(lite=lite)
    if in_maps is None:
        in_maps = _prep_in_maps(x, adj, W1, W2, b2)
    res = run_bass_kernel_spmd(nc, in_maps, core_ids=list(range(NCORE)),
                               trace=trace)
    out = np.concatenate([r["out"] for r in res.results], axis=0)
    return out, res


def kernel(x, adj, W1, W2, b2):
    out, _ = _run(x, adj, W1, W2, b2, trace=False)
    return out


# revision 12
# speedup vs baseline: 1.7273x; 1.1333x over previous
"""MidGCN forward on 8 Trainium2 NeuronCores (Bass/Tile, SPMD row-sharding).

Math (alpha = 0.5):
  DAD   = d_row * adj * d_col          (d = rsqrt of row/col sums)
  adj_f = (0.5*I - DAD)(I + DAD) = 0.5*I - 0.5*DAD - DAD@DAD
  h     = relu(adj_f @ (x @ W1))
  out   = log_softmax(adj_f @ (h @ W2) + b2)

Strategy vs the bf16 predecessor (269.5us):
  * The adj slab is resident in SBUF as fp8e4 (e4m3): halves the HBM
    load (8MB) and enables DoubleRow matmuls (two 128-deep k-tiles per
    instruction at 0.5 cycles/row): the two NxN passes drop ~4x.
  * d_col is folded into the narrow activations (z's), never into the
    slab: z_q = fp8(S * d_col * z) per pass, so the slab stays raw and
    single-quantized.  Scales S1/S2/Sv/Su keep each z at sigma ~0.7 in
    e4m3 range; the inverse scales fold into the per-row epilogue
    scalars.  Verified numerically: rel err ~5.7e-3 (tolerance 2e-2).
  * Colsum partials ride the slab DMA on DVE/ACT; rowsums via a
    DoubleRow fp8 ones-vector PE pass.  ReduceScatter (not AllReduce)
    delivers exactly the own-row d_col slice - no per-core addressing.
  * rsqrt = Exp(-0.5*Ln(x)): every ACT func used ({Copy,Relu,Exp,Ln})
    lives in one act table -> a single LoadActFuncSet for the kernel.
  * Each pass computes mt-halves 0-3 / 4-7 separately with per-half
    AllGathers; the consumer pass processes the k-parity of the first
    half first, hiding collective+DMA latency under live matmuls.
  * Epilogues use fused scalar_tensor_tensor (one DVE op per AXPY) and
    ACT Copy-with-scale for the fp8 quantizes straight out of PSUM.
"""

import numpy as np
import ml_dtypes

NCORE = 8
N = 8192
NF = 512
NH = 256
NC = 2
RPC = N // NCORE          # rows per core = 1024
KT = N // 128             # 64 contraction tiles
NQ = KT // 2              # 32 DoubleRow k-pairs
MT = RPC // 128           # 8 output row tiles per core
FT = NF // 128            # 4 k-tiles for x @ W1
HR = RPC // 2             # 512 rows per mt-half

S1 = 64.0                 # zq = fp8(S1 * d_col * zs)
S2 = 4096.0               # zt = fp8(psum1 * d_row*d_col*S2/S1)
SV = 16.0                 # zv = fp8(SV * d_col * y2)
SU = 512.0                # zu = fp8(psum_u * d_row*d_col*SU/SV)

_CACHE = {}


def _build(lite=False, sim=False):
    import concourse.bass as bass
    import concourse.mybir as mybir
    import concourse.tile as tile
    from concourse import bacc, masks
    from concourse.bass import ts

    BF = mybir.dt.bfloat16
    F32 = mybir.dt.float32
    FP8 = mybir.dt.float8e4
    AX = mybir.AxisListType
    OP = mybir.AluOpType
    AF = mybir.ActivationFunctionType
    PM = mybir.MatmulPerfMode

    nc = bacc.Bacc("TRN2", target_bir_lowering=False, debug=False,
                   num_devices=NCORE)

    adjT = nc.dram_tensor("adjT", [N, RPC], FP8, kind="ExternalInput")
    xT = nc.dram_tensor("xT", [NF, RPC], BF, kind="ExternalInput")
    w1 = nc.dram_tensor("w1", [NF, NH], BF, kind="ExternalInput")
    w2h = nc.dram_tensor("w2h", [NH, NC], BF, kind="ExternalInput")
    b2 = nc.dram_tensor("b2", [1, NC], F32, kind="ExternalInput")
    out = nc.dram_tensor("out", [RPC, NC], F32, kind="ExternalOutput")

    cs_in = nc.dram_tensor("cs_in", [N], F32)
    cs_rs = nc.dram_tensor("cs_rs", [RPC], F32)
    rs_dram = nc.dram_tensor("rs_dram", [RPC], F32)
    zq_in = nc.dram_tensor("zq_in", [RPC, NH], FP8)
    zq_out = nc.dram_tensor("zq_out", [N, NH], FP8, addr_space="Shared")
    zt_in = nc.dram_tensor("zt_in", [RPC, NH], FP8)
    zt_o = [nc.dram_tensor(f"zt_o{h}", [N // 2, NH], FP8,
                           addr_space="Shared") for h in range(2)]
    zv_in = nc.dram_tensor("zv_in", [RPC, NC], FP8)
    zv_o = [nc.dram_tensor(f"zv_o{h}", [N // 2, NC], FP8,
                           addr_space="Shared") for h in range(2)]
    zu_in = nc.dram_tensor("zu_in", [RPC, NC], FP8)
    zu_o = [nc.dram_tensor(f"zu_o{h}", [N // 2, NC], FP8,
                           addr_space="Shared") for h in range(2)]
    RG = [list(range(NCORE))]

    if lite:
        # I/O-identical null kernel: measures tunnel/dispatch overhead.
        with tile.TileContext(nc) as tc:
            with tc.tile_pool(name="p0", bufs=1) as p0:
                o = p0.tile([128, MT, NC], F32, tag="o")
                nc.vector.memset(o, 0.0)
                nc.sync.dma_start(
                    out=out[:].rearrange("(mt p) c -> p mt c", p=128), in_=o)
        nc.compile()
        return nc

    # k-pair order for passes consuming half-gathered z: the parity-0
    # pairs (kt%8 in 0..3) arrive with AllGather #1, parity-1 with #2.
    QORD = [q for q in range(NQ) if q % 4 < 2] + \
           [q for q in range(NQ) if q % 4 >= 2]

    def hslot(q):
        # (half, flat slot) of k-pair q in the per-half gather layout
        h = 0 if q % 4 < 2 else 1
        return h, 4 * (q // 4) + 2 * (q % 4) - 4 * h

    with tile.TileContext(nc) as tc:
        from contextlib import ExitStack
        with ExitStack() as ctx:
            p_one = ctx.enter_context(tc.tile_pool(name="p_one", bufs=1))
            p_rot = ctx.enter_context(tc.tile_pool(name="p_rot", bufs=2))

            # ---------- persistent SBUF ----------
            slab = p_one.tile([128, KT, RPC], FP8, tag="slab")
            zbq = p_one.tile([128, KT, NH], FP8, tag="zbq")
            zbt = [p_one.tile([128, KT // 2, NH], FP8, tag=f"zbt{h}",
                              name=f"zbt{h}") for h in range(2)]
            zvf = [p_one.tile([128, KT // 2, NC], FP8, tag=f"zvf{h}",
                              name=f"zvf{h}") for h in range(2)]
            zuf = [p_one.tile([128, KT // 2, NC], FP8, tag=f"zuf{h}",
                              name=f"zuf{h}") for h in range(2)]
            xT_sb = p_one.tile([128, FT, RPC], BF, tag="xT")
            w1_sb = p_one.tile([128, FT, NH], BF, tag="w1")
            w2_sb = p_one.tile([128, 2, NC], BF, tag="w2")
            b2_sb = p_one.tile([128, NC], F32, tag="b2")
            ident = p_one.tile([128, 128], BF, tag="ident")
            ones8 = p_one.tile([128, 2, 32], FP8, tag="ones")
            csp = p_one.tile([128, KT], F32, tag="csp")
            s_sb = p_one.tile([128, MT, NH], F32, tag="s")
            zq_sb = p_one.tile([128, MT, NH], FP8, tag="zq")
            zt_sb = p_one.tile([128, MT, NH], FP8, tag="zt")
            zv_sb = p_one.tile([128, MT, NC], FP8, tag="zv")
            zu_sb = p_one.tile([128, MT, NC], FP8, tag="zu")
            y_sb = p_one.tile([128, MT, NC], F32, tag="y")
            row_sb = p_one.tile([1, RPC], F32, tag="rowsb")
            rloc = p_one.tile([128, MT], F32, tag="rloc")
            drow = p_one.tile([128, MT], F32, tag="drow")
            dcl = p_one.tile([128, MT], F32, tag="dcl")
            ddt = p_one.tile([128, MT], F32, tag="ddt")
            q1 = p_one.tile([128, MT], F32, tag="q1")
            dd2 = p_one.tile([128, MT], F32, tag="dd2")
            dvq = p_one.tile([128, MT], F32, tag="dvq")
            du2 = p_one.tile([128, MT], F32, tag="du2")
            e1n = p_one.tile([128, MT], F32, tag="e1n")
            e2 = p_one.tile([128, MT], F32, tag="e2")
            eu = p_one.tile([128, MT], F32, tag="eu")
            ew = p_one.tile([128, MT], F32, tag="ew")
            mx_sb = p_one.tile([128, MT], F32, tag="mx")
            out_sb = p_one.tile([128, MT, NC], F32, tag="osb")

            masks.make_identity(nc, ident)
            nc.vector.memset(ones8, 1.0)

            # weights / x first on the DMA queue, then the slab
            nc.sync.dma_start(out=xT_sb, in_=xT[:].rearrange(
                "(kt p) m -> p kt m", p=128))
            nc.sync.dma_start(out=w1_sb, in_=w1[:].rearrange(
                "(kt p) n -> p kt n", p=128))
            nc.sync.dma_start(out=w2_sb, in_=w2h[:].rearrange(
                "(kh p) c -> p kh c", p=128))
            nc.sync.dma_start(out=b2_sb, in_=b2[:].to_broadcast([128, NC]))

            with ExitStack() as c1:
                pm = c1.enter_context(
                    tc.tile_pool(name="pm", bufs=4, space="PSUM"))

                # ---- s = x @ W1 first in PE program order ----
                for g in range(2):
                    mts = range(4 * g, 4 * g + 4)
                    ps = {mt: pm.tile([128, NH], F32, tag="pm",
                                      name=f"ps{mt}") for mt in mts}
                    for kt in range(FT):
                        for mt in mts:
                            nc.tensor.matmul(ps[mt], xT_sb[:, kt, ts(mt, 128)],
                                             w1_sb[:, kt, :],
                                             start=kt == 0, stop=kt == FT - 1)
                    for mt in mts:
                        nc.vector.tensor_copy(s_sb[:, mt, :], ps[mt])

                # ---- slab load; colsum partials on DVE/ACT; rowsum on PE --
                with ExitStack() as c0:
                    pr_p = c0.enter_context(
                        tc.tile_pool(name="prp", bufs=4, space="PSUM"))
                    prow = [pr_p.tile([32, 256], F32, tag="pr",
                            name=f"pr{j}") for j in range(4)]
                    for c in range(8):
                        nc.sync.dma_start(
                            out=slab[:, 8 * c:8 * c + 8, :],
                            in_=adjT[ts(c, RPC), :].rearrange(
                                "(kt p) m -> p kt m", p=128))
                        nd = 5 if c == 7 else 4
                        k0 = 8 * c
                        nc.vector.tensor_reduce(
                            out=csp[:, k0:k0 + nd],
                            in_=slab[:, k0:k0 + nd, :],
                            axis=AX.X, op=OP.add)
                        for kt in range(k0 + nd, k0 + 8):
                            scr = p_rot.tile([128, RPC], FP8, tag="scr",
                                             name=f"scr{kt}")
                            nc.scalar.activation(
                                out=scr, in_=slab[:, kt, :], func=AF.Copy,
                                accum_out=csp[:, kt:kt + 1])
                        for q in range(4 * c, 4 * c + 4):
                            for j in range(4):
                                nc.tensor.matmul(
                                    prow[j], ones8,
                                    slab[:, 2 * q:2 * q + 2, ts(j, 256)],
                                    start=q == 0, stop=q == NQ - 1,
                                    perf_mode=PM.DoubleRow)
                    for j in range(4):
                        nc.vector.tensor_copy(row_sb[0:1, ts(j, 256)],
                                              prow[j][0:1, :])

                # d_row/d_col = exp(-0.5 ln(sum)): group Ln's then Exp's
                # so the act table switches only twice.
                nc.sync.dma_start(out=rs_dram[:], in_=row_sb[0:1, :])
                nc.sync.dma_start(
                    out=rloc,
                    in_=rs_dram[:].rearrange("(mt p) -> p mt", p=128))

                # ---- colsum store via PE transpose (contiguous (kt p)) ----
                with ExitStack() as ct:
                    ptc = ct.enter_context(
                        tc.tile_pool(name="ptc", bufs=1, space="PSUM"))
                    identf = p_one.tile([128, 128], F32, tag="identf")
                    masks.make_identity(nc, identf)
                    ctp = ptc.tile([KT, 128], F32, tag="ctp")
                    nc.tensor.transpose(ctp, csp, identf)
                    cst = p_one.tile([KT, 128], F32, tag="cst")
                    nc.vector.tensor_copy(cst, ctp)
                nc.sync.dma_start(
                    out=cs_in[:].rearrange("(kt p) -> kt p", kt=KT), in_=cst)
                if sim:
                    nc.sync.dma_start(out=cs_rs[:], in_=cs_in[0:RPC])
                else:
                    nc.gpsimd.collective_compute(
                        "ReduceScatter", OP.add, replica_groups=RG,
                        ins=[cs_in[:]], outs=[cs_rs[:]])
                nc.sync.dma_start(
                    out=dcl, in_=cs_rs[:].rearrange("(mt p) -> p mt", p=128))
                nc.scalar.activation(out=drow, in_=rloc, func=AF.Ln)
                nc.scalar.activation(out=dcl, in_=dcl, func=AF.Ln)
                nc.scalar.activation(out=drow, in_=drow, func=AF.Exp,
                                     scale=-0.5)
                nc.scalar.activation(out=dcl, in_=dcl, func=AF.Exp,
                                     scale=-0.5)

                # per-row scalar vectors
                nc.vector.tensor_tensor(ddt, drow, dcl, op=OP.mult)
                nc.vector.tensor_scalar_mul(q1, dcl, S1)
                nc.vector.tensor_scalar_mul(dd2, ddt, S2 / S1)
                nc.vector.tensor_scalar_mul(dvq, dcl, SV)
                nc.vector.tensor_scalar_mul(du2, ddt, SU / SV)
                nc.vector.tensor_scalar_mul(e1n, drow, -1.0 / S1)
                nc.vector.tensor_scalar_mul(e2, drow, -2.0 / S2)
                nc.vector.tensor_scalar_mul(eu, drow, -0.5 / SV)
                nc.vector.tensor_scalar_mul(ew, drow, -1.0 / SU)

                # zq = fp8(S1 * d_col * zs); gather; load k-major
                for mt in range(MT):
                    nc.vector.tensor_scalar(zq_sb[:, mt, :], s_sb[:, mt, :],
                                            q1[:, mt:mt + 1], None,
                                            op0=OP.mult)
                nc.sync.dma_start(
                    out=zq_in[:].rearrange("(mt p) n -> p mt n", p=128),
                    in_=zq_sb)
                if sim:
                    nc.sync.dma_start(out=zq_out[0:RPC, :], in_=zq_in[:])
                else:
                    nc.gpsimd.collective_compute(
                        "AllGather", OP.bypass, replica_groups=RG,
                        ins=[zq_in[:]], outs=[zq_out[:]])
                for c in range(4):
                    nc.sync.dma_start(
                        out=zbq[:, 16 * c:16 * c + 16, :],
                        in_=zq_out[ts(c, 2048), :].rearrange(
                            "(kt p) n -> p kt n", p=128))

                # ---------- pass 1: psum1 = adj @ zq, by mt-halves ----------
                for hf in range(2):
                    mts = range(4 * hf, 4 * hf + 4)
                    pp = {mt: pm.tile([128, NH], F32, tag="pm",
                                      name=f"p1_{mt}") for mt in mts}
                    for q in range(NQ):
                        for mt in mts:
                            nc.tensor.matmul(
                                pp[mt], slab[:, 2 * q:2 * q + 2, ts(mt, 128)],
                                zbq[:, 2 * q:2 * q + 2, :],
                                start=q == 0, stop=q == NQ - 1,
                                perf_mode=PM.DoubleRow)
                    # zt = fp8(psum1 * dd2) -> store half -> gather half
                    for mt in mts:
                        nc.vector.tensor_scalar(zt_sb[:, mt, :], pp[mt],
                                                dd2[:, mt:mt + 1], None,
                                                op0=OP.mult)
                    nc.sync.dma_start(
                        out=zt_in[ts(hf, HR), :].rearrange(
                            "(mt p) n -> p mt n", p=128),
                        in_=zt_sb[:, 4 * hf:4 * hf + 4, :])
                    if sim:
                        nc.sync.dma_start(out=zt_o[hf][0:HR, :],
                                          in_=zt_in[ts(hf, HR), :])
                    else:
                        nc.gpsimd.collective_compute(
                            "AllGather", OP.bypass, replica_groups=RG,
                            ins=[zt_in[ts(hf, HR), :]], outs=[zt_o[hf][:]])
                    for cc in range(2):
                        nc.sync.dma_start(
                            out=zbt[hf][:, 16 * cc:16 * cc + 16, :],
                            in_=zt_o[hf][ts(cc, 2048), :].rearrange(
                                "(f p) n -> p f n", p=128))
                    # A = s - T = s + e1n * psum1 (in place in s_sb)
                    for mt in mts:
                        nc.vector.scalar_tensor_tensor(
                            s_sb[:, mt, :], pp[mt], e1n[:, mt:mt + 1],
                            s_sb[:, mt, :], op0=OP.mult, op1=OP.add)

                # ---------- pass 2 + layer-1 epilogue ----------
                with ExitStack() as c2:
                    ptr = c2.enter_context(
                        tc.tile_pool(name="ptr", bufs=2, space="PSUM"))
                    pv = c2.enter_context(
                        tc.tile_pool(name="pv", bufs=1, space="PSUM"))
                    for hf in range(2):
                        mts = range(4 * hf, 4 * hf + 4)
                        pp = {mt: pm.tile([128, NH], F32, tag="pm",
                                          name=f"p2_{mt}") for mt in mts}
                        for qi, q in enumerate(QORD):
                            for mt in mts:
                                h_, f_ = hslot(q)
                                nc.tensor.matmul(
                                    pp[mt],
                                    slab[:, 2 * q:2 * q + 2, ts(mt, 128)],
                                    zbt[h_][:, f_:f_ + 2, :],
                                    start=qi == 0, stop=qi == NQ - 1,
                                    perf_mode=PM.DoubleRow)
                        for mt in mts:
                            # h2 = relu(A + e2*psum2); y2 = h2 @ (W2/2)
                            h2p = p_rot.tile([128, NH], F32, tag="h2p",
                                             bufs=3)
                            nc.vector.scalar_tensor_tensor(
                                h2p, pp[mt], e2[:, mt:mt + 1], s_sb[:, mt, :],
                                op0=OP.mult, op1=OP.add)
                            hp = p_rot.tile([128, NH], BF, tag="hp", bufs=3)
                            nc.vector.tensor_scalar_max(hp, h2p, 0.0)
                            psv = pv.tile([128, NC], F32, tag="pv")
                            for kh in range(2):
                                pstr = ptr.tile([128, 128], BF, tag="ptr")
                                nc.tensor.transpose(pstr, hp[:, ts(kh, 128)],
                                                    ident)
                                hT = p_rot.tile([128, 128], BF, tag="hT",
                                                bufs=3)
                                nc.vector.tensor_copy(hT, pstr)
                                nc.tensor.matmul(psv, hT, w2_sb[:, kh, :],
                                                 start=kh == 0, stop=kh == 1)
                            nc.vector.tensor_scalar_mul(y_sb[:, mt, :],
                                                        psv, 0.5)
                            nc.vector.tensor_scalar(zv_sb[:, mt, :], psv,
                                                    dvq[:, mt:mt + 1], None,
                                                    op0=OP.mult)
                        nc.sync.dma_start(
                            out=zv_in[ts(hf, HR), :].rearrange(
                                "(mt p) c -> p mt c", p=128),
                            in_=zv_sb[:, 4 * hf:4 * hf + 4, :])
                        if sim:
                            nc.sync.dma_start(out=zv_o[hf][0:HR, :],
                                              in_=zv_in[ts(hf, HR), :])
                        else:
                            nc.gpsimd.collective_compute(
                                "AllGather", OP.bypass, replica_groups=RG,
                                ins=[zv_in[ts(hf, HR), :]],
                                outs=[zv_o[hf][:]])
                        nc.sync.dma_start(
                            out=zvf[hf],
                            in_=zv_o[hf][:].rearrange("(f p) c -> p f c",
                                                      p=128))

            # ---------- narrow passes ----------
            with ExitStack() as c3:
                pnar = c3.enter_context(
                    tc.tile_pool(name="pnar", bufs=8, space="PSUM"))

                # pass 3: psum_u = adj @ zv
                for hf in range(2):
                    mts = range(4 * hf, 4 * hf + 4)
                    pu = {mt: pnar.tile([128, NC], F32, tag="pu",
                                        name=f"pu{mt}") for mt in mts}
                    for qi, q in enumerate(QORD):
                        for mt in mts:
                            h_, f_ = hslot(q)
                            nc.tensor.matmul(
                                pu[mt], slab[:, 2 * q:2 * q + 2, ts(mt, 128)],
                                zvf[h_][:, f_:f_ + 2, :],
                                start=qi == 0, stop=qi == NQ - 1,
                                perf_mode=PM.DoubleRow)
                    for mt in mts:
                        nc.vector.tensor_scalar(zu_sb[:, mt, :], pu[mt],
                                                du2[:, mt:mt + 1], None,
                                                op0=OP.mult)
                    nc.sync.dma_start(
                        out=zu_in[ts(hf, HR), :].rearrange(
                            "(mt p) c -> p mt c", p=128),
                        in_=zu_sb[:, 4 * hf:4 * hf + 4, :])
                    if sim:
                        nc.sync.dma_start(out=zu_o[hf][0:HR, :],
                                          in_=zu_in[ts(hf, HR), :])
                    else:
                        nc.gpsimd.collective_compute(
                            "AllGather", OP.bypass, replica_groups=RG,
                            ins=[zu_in[ts(hf, HR), :]], outs=[zu_o[hf][:]])
                    nc.sync.dma_start(
                        out=zuf[hf],
                        in_=zu_o[hf][:].rearrange("(f p) c -> p f c",
                                                  p=128))
                    # y += eu * psum_u  (= 0.5*y2 - 0.5*DAD@y2 so far)
                    for mt in mts:
                        nc.vector.scalar_tensor_tensor(
                            y_sb[:, mt, :], pu[mt], eu[:, mt:mt + 1],
                            y_sb[:, mt, :], op0=OP.mult, op1=OP.add)

                # fold the bias in while the gather flies
                for mt in range(MT):
                    nc.vector.tensor_add(y_sb[:, mt, :], y_sb[:, mt, :],
                                         b2_sb)

                # pass 4: psum_w = adj @ zu ; G = y + ew*psum_w; log_softmax
                # (grouped epilogue: all DVE prep, then Exp block, Ln block)
                pw = {}
                for hf in range(2):
                    mts = range(4 * hf, 4 * hf + 4)
                    for mt in mts:
                        pw[mt] = pnar.tile([128, NC], F32, tag="pu",
                                           name=f"pw{mt}")
                    for qi, q in enumerate(QORD):
                        for mt in mts:
                            h_, f_ = hslot(q)
                            nc.tensor.matmul(
                                pw[mt], slab[:, 2 * q:2 * q + 2, ts(mt, 128)],
                                zuf[h_][:, f_:f_ + 2, :],
                                start=qi == 0, stop=qi == NQ - 1,
                                perf_mode=PM.DoubleRow)
                G_ts, sm_ts = {}, {}
                for mt in range(MT):
                    G_t = p_rot.tile([128, NC], F32, tag="G",
                                     name=f"G{mt}", bufs=8)
                    nc.vector.scalar_tensor_tensor(
                        G_t, pw[mt], ew[:, mt:mt + 1], y_sb[:, mt, :],
                        op0=OP.mult, op1=OP.add)
                    mx_t = p_rot.tile([128, 1], F32, tag="mx")
                    nc.vector.tensor_reduce(out=mx_t, in_=G_t,
                                            axis=AX.X, op=OP.max)
                    nc.vector.tensor_scalar(G_t, G_t, mx_t, None,
                                            op0=OP.subtract)
                    G_ts[mt] = G_t
                for mt in range(MT):
                    ex_t = p_rot.tile([128, NC], F32, tag="ex")
                    sm_t = p_rot.tile([128, 1], F32, tag="sm",
                                      name=f"sm{mt}", bufs=8)
                    nc.scalar.activation(out=ex_t, in_=G_ts[mt],
                                         func=AF.Exp, accum_out=sm_t)
                    sm_ts[mt] = sm_t
                for mt in range(MT):
                    lg_t = p_rot.tile([128, 1], F32, tag="lg")
                    nc.scalar.activation(out=lg_t, in_=sm_ts[mt],
                                         func=AF.Ln)
                    nc.vector.tensor_scalar(out_sb[:, mt, :], G_ts[mt],
                                            lg_t, None, op0=OP.subtract)
                nc.sync.dma_start(
                    out=out[:].rearrange("(mt p) c -> p mt c", p=128),
                    in_=out_sb)

    nc.compile()
    return nc


def _get_nc(lite=False):
    key = "nc_lite" if lite else "nc"
    if key not in _CACHE:
        _CACHE[key] = _build(lite=lite)
    return _CACHE[key]


def _prep_in_maps(x, adj, W1, W2, b2):
    bf = ml_dtypes.bfloat16
    e4 = ml_dtypes.float8_e4m3fn
    f32 = np.float32
    x = np.asarray(x, f32)
    adj = np.asarray(adj, f32)
    w1 = np.asarray(W1, f32).astype(bf)
    w2h = (0.5 * np.asarray(W2, f32)).astype(bf)
    b2v = np.asarray(b2, f32).reshape(1, NC)
    in_maps = []
    for i in range(NCORE):
        rows = slice(i * RPC, (i + 1) * RPC)
        in_maps.append({
            "adjT": adj[rows, :].T.astype(e4),   # one fused copy+cast
            "xT": x[rows, :].T.astype(bf),
            "w1": w1, "w2h": w2h, "b2": b2v,
        })
    return in_maps


def _run(x, adj, W1, W2, b2, trace=False, lite=False, in_maps=None):
    from concourse.bass_utils import run_bass_kernel_spmd
    nc = _get_nc(lite=lite)
    if in_maps is None:
        in_maps = _prep_in_maps(x, adj, W1, W2, b2)
    res = run_bass_kernel_spmd(nc, in_maps, core_ids=list(range(NCORE)),
                               trace=trace)
    out = np.concatenate([r["out"] for r in res.results], axis=0)
    return out, res


def kernel(x, adj, W1, W2, b2):
    out, _ = _run(x, adj, W1, W2, b2, trace=False)
    return out
